# revision 37
# baseline (speedup 1.0000x reference)
"""Trainium2 Bass kernel: batched causal attention (B=4, S=4096, E=256, f32).

Sharding: 2 cores per batch element (4 pairs). Within a pair, the key/value
rows are split even/odd at 128-row tile granularity; both cores process all
4096 query rows of their batch against their 2048 K/V rows.  This keeps the
SPMD instruction stream identical across cores (only data differs) and
perfectly load-balances the causal structure.  Partial (P@V, rowsum) results
are merged across each pair with per-pair ReduceScatters (bf16 payload),
after which each core normalizes and writes half the batch rows.

v3 layout:
  - X^T, Z^T, W^T pre-transposed and pre-cast to bf16 on the host: no PE
    transposes / DVE copies on the load path, large contiguous DMAs.
  - All post-phase work is issued AFTER all attention work + collectives:
    strict-FIFO engine/DMA queues never stall attention behind an
    RS-dependent instruction (this was worth ~30us of mid-kernel craters).
  - Rowsums via per-k-tile ones-matmul accumulated in PSUM (PE has slack;
    Vector/GpSimd accumulate variants stall the pipeline).
  - Consts packed into two DMAs; loads split across the two HWDGE queues.
"""

import numpy as np

B = 4
S = 4096
E = 256
SK = S // 2          # K/V rows per core
KT = SK // 128       # 16 local k-tiles
NCHUNK = S // 512    # 8 q-chunks of 512
F = 512              # q free dim per chunk
NPOST = NCHUNK // 2  # post-phase chunks per core

_COMPILED = {}


def _build():
    import concourse.bass as bass
    import concourse.tile as tile
    from concourse import mybir, bacc

    f32 = mybir.dt.float32
    bf16 = mybir.dt.bfloat16
    Exp = mybir.ActivationFunctionType.Exp
    Copy = mybir.ActivationFunctionType.Copy
    Ident = mybir.ActivationFunctionType.Identity
    Alu = mybir.AluOpType

    nc = bacc.Bacc("TRN2", target_bir_lowering=False, debug=False,
                   enable_asserts=True, num_devices=8)

    xT_ext = nc.dram_tensor("xt", [E, S], bf16, kind="ExternalInput")
    zT_ext = nc.dram_tensor("zt", [E, SK], bf16, kind="ExternalInput")
    wqT_ext = nc.dram_tensor("wqt", [E, E], bf16, kind="ExternalInput")
    wkT_ext = nc.dram_tensor("wkt", [E, E], bf16, kind="ExternalInput")
    wvT_ext = nc.dram_tensor("wvt", [E, E], bf16, kind="ExternalInput")
    # packed consts: bf16 [128, 1280] = masks(2x512) | ones(128) | ident(128)
    cb_ext = nc.dram_tensor("cb", [128, 1280], bf16, kind="ExternalInput")
    # packed consts: f32 [128, 258] = bqs(2) | bv broadcast(256)
    cf_ext = nc.dram_tensor("cf", [128, 258], f32, kind="ExternalInput")
    out_ext = nc.dram_tensor("out", [S // 2, E], f32, kind="ExternalOutput")

    with tile.TileContext(nc) as tc:
        with tc.tile_pool(name="singles", bufs=1) as singles, \
             tc.tile_pool(name="dram", bufs=1, space="DRAM") as dram:
            # ---- weights + Z^T interleaved in need-order on the scalar
            # HWDGE ring (it comes up first): the first K-proj matmul needs
            # exactly wk + zT chunk 0.
            zT = singles.tile([128, 2, SK], bf16, tag="zT")

            def load_zt(sc, eng):
                eng.dma_start(
                    out=zT[:, :, F * sc:F * (sc + 1)],
                    in_=zT_ext[:, F * sc:F * (sc + 1)].rearrange(
                        "(c p) s -> p c s", p=128))

            wT = {}
            for wname, wext in (("k", wkT_ext), ("v", wvT_ext), ("q", wqT_ext)):
                wT[wname] = singles.tile([128, 2, E], bf16, name=f"wT_{wname}")
            nc.scalar.dma_start(
                out=wT["k"][:], in_=wkT_ext.ap().rearrange("(c p) f -> p c f", p=128))
            load_zt(0, nc.scalar)
            nc.scalar.dma_start(
                out=wT["v"][:], in_=wvT_ext.ap().rearrange("(c p) f -> p c f", p=128))
            load_zt(1, nc.scalar)
            nc.scalar.dma_start(
                out=wT["q"][:], in_=wqT_ext.ap().rearrange("(c p) f -> p c f", p=128))
            load_zt(2, nc.scalar)
            cf = singles.tile([128, 258], f32)
            nc.scalar.dma_start(out=cf[:], in_=cf_ext[:])
            load_zt(3, nc.scalar)
            cb = singles.tile([128, 1280], bf16)
            nc.scalar.dma_start(out=cb[:], in_=cb_ext[:])
            maskt = cb[:, 0:1024].rearrange("p (m f) -> p m f", m=2)
            ones_r = cb[:, 1024:1152]
            ident_bf = cb[:, 1152:1280]
            bqs = cf[:, 0:2]
            bv_bc = cf[:, 2:258]

            # ---- big persistent SBUF tensors -------------------------------
            qT = singles.tile([128, 2, S], bf16, tag="qT")
            kT = singles.tile([128, 2, SK], bf16, tag="kT")
            xTt = singles.tile([128, 2, S], bf16, tag="xTt")
            v_sb = singles.tile([128, KT, E], bf16, tag="v_sb")

            # separate DRAM tiles per pair: a shared tile would create a
            # coarse write-after-read dep of pair p-1's partial writes on
            # pair p's ReduceScatter, stalling the DMA queue and the PE
            partials_in = [dram.tile([2, 257, F], bf16, name=f"pin{p}")
                           for p in range(NPOST)]
            partials_out = [dram.tile([257, F], bf16, name=f"pout{p}")
                            for p in range(NPOST)]

            with tc.tile_pool(name="ps_mm", bufs=3, space="PSUM") as ps_mm:
                # K^T and V from Z^T (own k-parity half)
                for sc in range(4):
                    for ft in range(2):
                        psk = ps_mm.tile([128, F], f32, tag="ps_mm", name="psk")
                        for et in range(2):
                            nc.tensor.matmul(psk[:], wT["k"][:, et, 128 * ft:128 * (ft + 1)],
                                             zT[:, et, F * sc:F * (sc + 1)],
                                             start=(et == 0), stop=(et == 1))
                        nc.vector.tensor_copy(out=kT[:, ft, F * sc:F * (sc + 1)],
                                              in_=psk[:])
                    for t in range(4):
                        psv = ps_mm.tile([128, E], f32, tag="ps_mm", name="psv",
                                         padded_shape=[128, 512])
                        for et in range(2):
                            nc.tensor.matmul(psv[:],
                                             zT[:, et, F * sc + 128 * t:F * sc + 128 * (t + 1)],
                                             wT["v"][:, et, :],
                                             start=(et == 0), stop=(et == 1))
                        nc.vector.tensor_copy(out=v_sb[:, 4 * sc + t, :], in_=psv[:])

                # Q^T from X^T (all q rows), chunk order = attention order
                for i, j in enumerate((0, 4, 1, 5, 2, 6, 3, 7)):
                    eng = nc.sync if i % 2 == 0 else nc.scalar
                    eng.dma_start(
                        out=xTt[:, :, F * j:F * (j + 1)],
                        in_=xT_ext[:, F * j:F * (j + 1)].rearrange(
                            "(c p) s -> p c s", p=128))
                    for ft in range(2):
                        psq = ps_mm.tile([128, F], f32, tag="ps_mm", name="psq")
                        for et in range(2):
                            nc.tensor.matmul(psq[:], wT["q"][:, et, 128 * ft:128 * (ft + 1)],
                                             xTt[:, et, F * j:F * (j + 1)],
                                             start=(et == 0), stop=(et == 1))
                        nc.scalar.activation(out=qT[:, ft, F * j:F * (j + 1)],
                                             in_=psq[:], func=Ident,
                                             bias=bqs[:, ft:ft + 1],
                                             scale=1.0 / float(np.sqrt(E)))

            with tc.tile_pool(name="pT", bufs=8) as pTp, \
                 tc.tile_pool(name="accp", bufs=2) as accp, \
                 tc.tile_pool(name="partsb", bufs=4) as partsb, \
                 tc.tile_pool(name="post", bufs=2) as post, \
                 tc.tile_pool(name="ps_s", bufs=3, space="PSUM") as ps_s, \
                 tc.tile_pool(name="ps_o", bufs=2, space="PSUM") as ps_o, \
                 tc.tile_pool(name="ps_rs", bufs=1, space="PSUM") as ps_rs:

                def attn_chunk(j, pair, half):
                    # rowsum accumulates OFF the PE: Vector takes even
                    # k-tiles, GpSimd odd ones (each engine owns one
                    # accumulator, chains stay short).  The partition
                    # reduction (ones-matmul) is emitted by finish_rowsum()
                    # one chunk later so it never stalls the PE FIFO.
                    nkt = 2 * (j + 1)
                    pso = ps_o.tile([128, 2 * F], f32, tag="ps_o", name="pso")
                    acc0 = accp.tile([128, F], f32, tag="acc0", name="acc0")
                    acc1 = accp.tile([128, F], f32, tag="acc1", name="acc1")
                    for ll in range(nkt):
                        pss = ps_s.tile([128, F], f32, tag="ps_s", name="pss")
                        for et in range(2):
                            nc.tensor.matmul(pss[:], kT[:, et, 128 * ll:128 * (ll + 1)],
                                             qT[:, et, F * j:F * (j + 1)],
                                             start=(et == 0), stop=(et == 1))
                        pT = pTp.tile([128, F], bf16, tag="pT", name="pT")
                        nc.scalar.activation(out=pT[:], in_=pss[:], func=Exp)
                        if ll >= nkt - 2:
                            nc.vector.tensor_mul(pT[:], pT[:],
                                                 maskt[:, ll - (nkt - 2), :])
                        for ft in range(2):
                            nc.tensor.matmul(pso[:, F * ft:F * (ft + 1)],
                                             v_sb[:, ll, 128 * ft:128 * (ft + 1)],
                                             pT[:], start=(ll == 0), stop=(ll == nkt - 1),
                                             skip_group_check=True)
                        if ll == 0:
                            nc.vector.tensor_copy(out=acc0[:], in_=pT[:])
                        elif ll == 1:
                            nc.gpsimd.tensor_copy(out=acc1[:], in_=pT[:])
                        elif ll % 2 == 0:
                            nc.vector.tensor_add(acc0[:], acc0[:], pT[:])
                        else:
                            nc.gpsimd.tensor_add(acc1[:], acc1[:], pT[:])
                    po_sb = partsb.tile([128, 2 * F], bf16, tag="po_sb", name="po_sb")
                    # split PSUM evacuation across Scalar and Vector: halves
                    # the pso hold time so the next chunk's PV can start
                    nc.scalar.activation(out=po_sb[:, 0:F], in_=pso[:, 0:F], func=Copy)
                    nc.vector.tensor_copy(out=po_sb[:, F:2 * F], in_=pso[:, F:2 * F])
                    nc.sync.dma_start(
                        out=partials_in[pair][half, 0:256, :].rearrange(
                            "(c p) f -> p c f", p=128),
                        in_=po_sb[:].rearrange("p (c f) -> p c f", c=2))
                    return acc0, acc1

                def finish_rowsum(accs, pair, half):
                    acc0, acc1 = accs
                    acc_bf = accp.tile([128, F], bf16, tag="accbf", name="acc_bf")
                    nc.vector.tensor_add(acc_bf[:], acc0[:], acc1[:])
                    psr = ps_rs.tile([128, F], f32, tag="ps_rs", name="psr")
                    nc.tensor.matmul(psr[:], ones_r[:], acc_bf[:],
                                     start=True, stop=True)
                    pr_sb = partsb.tile([1, F], bf16, tag="pr_sb", name="pr_sb")
                    nc.vector.tensor_copy(out=pr_sb[:], in_=psr[0:1, :])
                    nc.sync.dma_start(out=partials_in[pair][half, 256, :], in_=pr_sb[0:1, :])

                def post_chunk(c):
                    oT_sb = post.tile([128, 2 * F], bf16, tag="oT_sb", name="oT_sb")
                    nc.scalar.dma_start(
                        out=oT_sb[:].rearrange("p (c f) -> p c f", c=2),
                        in_=partials_out[c % NPOST][0:256, :].rearrange("(c p) f -> p c f", p=128))
                    rs_ld = post.tile([128, 4], bf16, tag="rs_ld", name="rs_ld")
                    nc.scalar.dma_start(out=rs_ld[:],
                                        in_=partials_out[c % NPOST][256, :].rearrange("(t p) -> p t", p=128))
                    rs_t = post.tile([128, 4], f32, tag="rs_t", name="rs_t")
                    nc.vector.reciprocal(out=rs_t[:], in_=rs_ld[:])
                    onat = post.tile([128, 4, E], f32, tag="onat", name="onat")
                    for t in range(4):
                        pst = ps_s.tile([128, E], bf16, tag="ps_s", name="pstp",
                                        padded_shape=[128, 1024])
                        for ft in range(2):
                            nc.tensor.transpose(
                                pst[:, 128 * ft:128 * (ft + 1)],
                                oT_sb[:, F * ft + 128 * t:F * ft + 128 * (t + 1)],
                                ident_bf[:])
                        nc.vector.scalar_tensor_tensor(
                            out=onat[:, t, :], in0=pst[:], scalar=rs_t[:, t:t + 1],
                            in1=bv_bc[:], op0=Alu.mult, op1=Alu.add)
                    nc.sync.dma_start(
                        out=out_ext[512 * c:512 * (c + 1), :].rearrange(
                            "(t p) e -> p t e", p=128),
                        in_=onat[:])

                # smallest pair first: the CC engine serializes the four
                # ReduceScatters (~15us each incl. an ~8us mesh rendezvous);
                # spreading them from early in the attention phase leaves
                # only the final pair's RS exposed at the end.  Each chunk's
                # rowsum reduction is deferred one chunk (but still emitted
                # before the RS that reads it).
                def emit_rs(pair):
                    nc.gpsimd.collective_compute(
                        "ReduceScatter", mybir.AluOpType.add,
                        replica_groups=[[0, 1], [2, 3], [4, 5], [6, 7]],
                        ins=[partials_in[pair].opt()],
                        outs=[partials_out[pair].opt()])

                pend = None  # (accs, pair, half) awaiting finish_rowsum
                rs_pend = None
                for pair in (0, 1, 2, 3):
                    for half, j in enumerate((pair, NPOST + pair)):
                        accs = attn_chunk(j, pair, half)
                        if pend is not None:
                            finish_rowsum(*pend)
                            if pend[2] == 1:
                                rs_pend = pend[1]
                        if rs_pend is not None:
                            emit_rs(rs_pend)
                            rs_pend = None
                        pend = (accs, pair, half)
                finish_rowsum(*pend)
                emit_rs(3)
                # scheduler fence: without it the scheduler backfills idle
                # engine slots mid-attention with post work that waits on a
                # ReduceScatter -> strict-FIFO head-of-line block starves the
                # PE for ~10us per pair (measured).  With the fence, posts
                # only overlap the final RS wait.
                tc.no_sync_barrier()
                for pair in (0, 1, 2, 3):
                    post_chunk(pair)

    nc.compile()
    return nc


def _get_nc():
    if "nc" not in _COMPILED:
        _COMPILED["nc"] = _build()
    return _COMPILED["nc"]


def kernel(X, Z, mask, Wq, bq, Wk, bk, Wv, bv):
    X = np.asarray(X, dtype=np.float32)
    Z = np.asarray(Z, dtype=np.float32)
    mask_np = np.asarray(mask)

    causal = bool(np.array_equal(
        mask_np != 0, np.tril(np.ones((S, S), dtype=bool))))
    if not causal:
        return _numpy_ref(X, Z, mask_np, Wq, bq, Wk, bk, Wv, bv)

    from concourse.bass_utils import run_bass_kernel_spmd

    nc = _get_nc()

    import ml_dtypes
    bf = ml_dtypes.bfloat16

    wqT = np.ascontiguousarray(np.asarray(Wq, dtype=np.float32).T).astype(bf)
    wkT = np.ascontiguousarray(np.asarray(Wk, dtype=np.float32).T).astype(bf)
    wvT = np.ascontiguousarray(np.asarray(Wv, dtype=np.float32).T).astype(bf)
    bqs = np.asarray(bq, dtype=np.float32) / np.float32(np.sqrt(E))
    bv = np.asarray(bv, dtype=np.float32)

    # f32 consts: bqs split in 2 cols of 128, bv broadcast to all partitions
    cf = np.empty((128, 258), dtype=np.float32)
    cf[:, 0] = bqs[:128]
    cf[:, 1] = bqs[128:]
    cf[:, 2:] = bv[None, :]

    # bf16 consts per parity: masks (last-2 local k-tiles), ones, identity
    y = np.arange(F)[None, :]
    x = np.arange(128)[:, None]
    cb_par = []
    for p in range(2):
        cbp = np.empty((128, 1280), dtype=np.float32)
        cbp[:, 0:512] = (y >= x + 128 * p)
        cbp[:, 512:1024] = (y >= x + 256 + 128 * p)
        cbp[:, 1024:1152] = 1.0
        cbp[:, 1152:1280] = np.eye(128, dtype=np.float32)
        cb_par.append(cbp.astype(bf))

    xT_by_batch = [np.ascontiguousarray(X[b].T).astype(bf) for b in range(B)]

    in_maps = []
    for c in range(8):
        b, p = c // 2, c % 2
        zb = Z[b].reshape(S // 128, 128, E)
        z_shard = zb[p::2].reshape(SK, E)
        zT_shard = np.ascontiguousarray(z_shard.T).astype(bf)
        in_maps.append({
            "xt": xT_by_batch[b],
            "zt": zT_shard,
            "wqt": wqT, "wkt": wkT, "wvt": wvT,
            "cb": cb_par[p],
            "cf": cf,
        })

    res = run_bass_kernel_spmd(nc, in_maps, core_ids=list(range(8)))

    out = np.empty((B, S, E), dtype=np.float32)
    for b in range(B):
        out[b, :S // 2] = res.results[2 * b]["out"]
        out[b, S // 2:] = res.results[2 * b + 1]["out"]
    return out


def _numpy_ref(X, Z, mask, Wq, bq, Wk, bk, Wv, bv):
    q = np.einsum("bse,fe->bsf", X, Wq) + bq
    k = np.einsum("bse,fe->bsf", Z, Wk) + bk
    v = np.einsum("bse,fe->bsf", Z, Wv) + bv
    s = np.einsum("bqe,bke->bqk", q, k) / np.sqrt(np.float32(X.shape[-1]))
    s = np.where(mask == 0, -np.inf, s)
    s = s - s.max(axis=-1, keepdims=True)
    p = np.exp(s)
    p /= p.sum(axis=-1, keepdims=True)
    return np.einsum("bqk,bke->bqe", p, v).astype(np.float32)
